# revision 19
# baseline (speedup 1.0000x reference)
"""GAT layer kernel for Trainium2 (8 NeuronCores, Bass/Tile).

Strategy:
  - Nodes are permuted by in-degree (host-side index preprocessing) so that
    128-node tiles have homogeneous degree; tiles are dealt round-robin to the
    8 cores so per-core edge counts balance and all cores share one SPMD
    instruction stream (per-tile padded degree K_r identical across cores).
  - Phase A (on device): table[n] = [seq_fts(n) (128, fp16) | f1(n)+b1 |
    f2(n)+b2] built with one PE matmul chain per 128-node tile against the
    host-side augmented weight matrix W_aug = [W | W@a1 | W@a2]; bias adds
    fused into the psum->fp16 copy; table written to HBM 4 tiles per DMA.
  - Phase B (on device): per node-tile, one indirect DMA per padded edge slot
    (plus a self slot providing f1) gathers table rows into a dense
    [node-partition x slot] SBUF layout.  SWDGE descriptor emission on the
    gpsimd Q7 (~8ns/row) is the hard bottleneck, so everything else hides
    under it.  Softmax over slots is free-dim work: ACT lrelu pieces -> ACT
    Exp with accum_out (denominator for free) -> one broadcast tensor_tensor
    multiply in place on the gathered tile -> fold-halves fp16 add tree ->
    normalize-after-aggregation -> fused elu(elu(x)).
    Dummy slots point at a table row with f2 = -60000 so exp() == 0 exactly.
"""

import os
import numpy as np

# ---- problem constants (self-contained; must match reference.py) ----
N_NODES = 100000
N_EDGES = 1600000
IN_DIM = 256
OUT_DIM = 128
LRELU_ALPHA = 0.2

NCORES = 8
P = 128
ROW = OUT_DIM + 2  # 128 feats + f1 + f2
DUMMY_F2 = -60000.0

_last_results = {}


def _ceil_to(x, m):
    return (x + m - 1) // m * m


def _preprocess(dst, src, n, npad):
    """Pure index preprocessing: degree-sort permutation, per-round padded
    degree K_r (exact max), and per-core gather index arrays."""
    ntiles = npad // P
    R = ntiles // NCORES

    deg = np.bincount(dst, minlength=npad).astype(np.int64)
    order = np.argsort(-deg, kind="stable")          # permuted pos -> node
    invpos = np.empty(npad, dtype=np.int64)
    invpos[order] = np.arange(npad)                  # node -> permuted pos

    posdeg = deg[order]                              # descending
    Kr = posdeg[np.arange(R) * (NCORES * P)].astype(np.int64)
    Sr = np.where(Kr > 0, Kr + 1, 0)                 # + self slot if nonempty
    offs = np.zeros(R + 1, dtype=np.int64)
    np.cumsum(P * Sr, out=offs[1:])
    TOT = int(offs[-1])

    # slot index of each edge within its destination node
    pos_d = invpos[dst]
    ordE = np.argsort(pos_d, kind="stable")
    pd_s = pos_d[ordE]
    sp_s = invpos[src][ordE]
    _, first, counts = np.unique(pd_s, return_index=True, return_counts=True)
    slot = np.arange(len(pd_s), dtype=np.int64) - np.repeat(first, counts)

    # SBUF-friendly layout: [128, CW] where column coff[r]+k = round r slot k
    coffs = np.zeros(R + 1, dtype=np.int64)
    np.cumsum(Sr, out=coffs[1:])
    CW = int(coffs[-1])

    g = pd_s >> 7
    p = pd_s & 127
    c = (g % NCORES).astype(np.int64)
    r = g // NCORES
    col = coffs[r] + slot

    idx_all = np.full((NCORES, P, max(CW, 1)), npad, dtype=np.int32)
    idx_all[c, p, col] = sp_s.astype(np.int32)

    # self slots: column coff[r] + K_r = own permuted position
    ne = Sr > 0
    rr = np.repeat(np.arange(R)[ne], P)
    pp = np.tile(np.arange(P), int(ne.sum()))
    self_col = coffs[rr] + Kr[rr]
    for cc in range(NCORES):
        own_pos = (rr * NCORES + cc) * P + pp
        idx_all[cc, pp, self_col] = own_pos.astype(np.int32)

    return order, Kr.tolist(), coffs, CW, idx_all


def _build_program(npad, Kr, coffs, CW, in_dim, exp_shift):
    import concourse.bass as bass
    import concourse.tile as tile
    from concourse import bacc, mybir
    from contextlib import ExitStack

    f16 = mybir.dt.float16
    f32 = mybir.dt.float32
    i32 = mybir.dt.int32
    AF = mybir.ActivationFunctionType
    OP = mybir.AluOpType
    D = OUT_DIM
    KT = in_dim // P
    R = len(Kr)
    rows_per_core = R * P

    nc = bacc.Bacc("TRN2", target_bir_lowering=False, debug=False,
                   num_devices=NCORES)
    xt_h = nc.declare_dram_parameter("xt", [in_dim, npad], f16, isOutput=False)
    waug_h = nc.declare_dram_parameter("waug", [in_dim, ROW], f16,
                                       isOutput=False)
    brow_h = nc.declare_dram_parameter("brow", [1, ROW], f32, isOutput=False)
    bias_h = nc.declare_dram_parameter("bias1", [1, D], f32, isOutput=False)
    dummy_h = nc.declare_dram_parameter("dumrow", [1, ROW], f16, isOutput=False)
    sidx_h = nc.declare_dram_parameter("sidx", [P, max(CW, 1)], i32,
                                       isOutput=False)
    out_h = nc.declare_dram_parameter("out", [rows_per_core, D], f32,
                                      isOutput=True)

    table_h = nc.dram_tensor("table", [npad + 1, ROW], f16)

    with tile.TileContext(nc) as tc, ExitStack() as ctx:
        cpool = ctx.enter_context(tc.tile_pool(name="consts", bufs=1))
        w_sb = [cpool.tile([P, ROW], f16, name=f"wsb{k}", tag=f"w{k}")
                for k in range(KT)]
        for k in range(KT):
            nc.sync.dma_start(out=w_sb[k][:], in_=waug_h[k * P:(k + 1) * P, :])
        brow_sb = cpool.tile([P, ROW], f32, tag="brow")
        nc.sync.dma_start(out=brow_sb[:],
                          in_=brow_h[0:1, :].to_broadcast([P, ROW]))
        bias_sb = cpool.tile([P, D], f32, tag="bias128")
        nc.sync.dma_start(out=bias_sb[:],
                          in_=bias_h[0:1, :].to_broadcast([P, D]))
        esh_sb = cpool.tile([P, 1], f32, tag="eshcol")
        nc.vector.memset(esh_sb[:], -float(exp_shift))
        zero_sb = cpool.tile([P, 1], f32, tag="zerocol")
        nc.vector.memset(zero_sb[:], 0.0)
        negone_sb = cpool.tile([P, 1], f32, tag="negonecol")
        nc.vector.memset(negone_sb[:], -1.0)
        dm_sb = cpool.tile([1, ROW], f16, tag="dummy")
        nc.sync.dma_start(out=dm_sb[:], in_=dummy_h[:, :])

        # fin0 = elu(elu(bias)) for empty rounds
        ob0 = cpool.tile([P, D], f16, tag="ob0")
        nc.vector.tensor_copy(out=ob0[:], in_=bias_sb[:])
        mm0 = cpool.tile([P, D], f16, tag="mm0")
        nc.vector.tensor_scalar(out=mm0[:], in0=ob0[:], scalar1=0.0,
                                scalar2=None, op0=OP.min)
        ex0 = cpool.tile([P, D], f16, tag="ex0")
        nc.scalar.activation(out=ex0[:], in_=mm0[:], func=AF.Exp,
                             bias=zero_sb[:, 0:1], scale=1.0)
        ex20 = cpool.tile([P, D], f16, tag="ex20")
        nc.scalar.activation(out=ex20[:], in_=ex0[:], func=AF.Exp,
                             bias=negone_sb[:, 0:1], scale=1.0)
        e10 = cpool.tile([P, D], f16, tag="e10")
        nc.vector.tensor_scalar(out=e10[:], in0=ex20[:], scalar1=-1.0,
                                scalar2=None, op0=OP.add)
        fin0 = cpool.tile([P, D], f32, tag="fin0")
        nc.vector.tensor_tensor(out=fin0[:], in0=ob0[:], in1=e10[:], op=OP.max)

        # ---------------- Phase A: build the table (node-major) ----------
        with nc.named_scope("phaseA"), ExitStack() as actx:
            xpool = actx.enter_context(tc.tile_pool(name="x", bufs=6))
            pspool = actx.enter_context(
                tc.tile_pool(name="psA", bufs=6, space="PSUM"))
            vpool = actx.enter_context(tc.tile_pool(name="vtile", bufs=4))

            ntile_all = npad // P
            for tb in range(ntile_all // 4):
                xks = []
                for k in range(KT):
                    xk = xpool.tile([P, 4 * P], f16, tag="xk", name=f"xk{k}")
                    nc.scalar.dma_start(
                        out=xk[:],
                        in_=xt_h[k * P:(k + 1) * P, tb * 4 * P:(tb + 1) * 4 * P])
                    xks.append(xk)
                vt4 = vpool.tile([P, 4 * ROW], f16, tag="vt4")
                for j in range(4):
                    ps = pspool.tile([P, ROW], f32, tag="ps")
                    for k in range(KT):
                        lhs = xks[k][:, j * P:(j + 1) * P]
                        nc.tensor.matmul(out=ps[:], lhsT=lhs, rhs=w_sb[k][:],
                                         start=(k == 0), stop=(k == KT - 1))
                    nc.vector.tensor_tensor(out=vt4[:, j * ROW:(j + 1) * ROW],
                                            in0=ps[:], in1=brow_sb[:],
                                            op=OP.add)
                nc.sync.dma_start(
                    out=table_h[tb * 4 * P:(tb + 1) * 4 * P, :].rearrange(
                        "(j p) w -> p j w", p=P),
                    in_=vt4[:].rearrange("p (j w) -> p j w", w=ROW))

        nc.sync.dma_start(out=table_h[npad:npad + 1, :], in_=dm_sb[:])
        tc.strict_bb_all_engine_barrier()

        # ---------------- Phase B: per node-tile edge processing ---------
        with nc.named_scope("phaseB"), ExitStack() as bctx:
            Kmax = max(Kr)
            Smax = Kmax + 1
            gbytes = Smax * ROW * 2
            gbufs = max(2, min(4, 110 * 1024 // gbytes))
            ipool = bctx.enter_context(tc.tile_pool(name="idx", bufs=1))
            gpool = bctx.enter_context(tc.tile_pool(name="g", bufs=gbufs))
            lpool = bctx.enter_context(tc.tile_pool(name="lr", bufs=3))
            epool = bctx.enter_context(tc.tile_pool(name="ee", bufs=3))
            spool = bctx.enter_context(tc.tile_pool(name="small", bufs=8))
            rpool = bctx.enter_context(tc.tile_pool(name="red", bufs=3))
            opool = bctx.enter_context(tc.tile_pool(name="on", bufs=4))
            fpool2 = bctx.enter_context(tc.tile_pool(name="fin", bufs=3))

            # preload the whole index array once; per-round gathers slice it
            idxall = ipool.tile([P, max(CW, 1)], i32, tag="idxall")
            nc.sync.dma_start(out=idxall[:, :], in_=sidx_h[:, :])

            for r in range(R):
                K = Kr[r]
                if K == 0:
                    nc.sync.dma_start(out=out_h[r * P:(r + 1) * P, :],
                                      in_=fin0[:])
                    continue
                S = K + 1
                co = int(coffs[r])
                G = gpool.tile([P, Smax * ROW], f16, tag="g")
                for k in range(S):
                    nc.gpsimd.indirect_dma_start(
                        out=G[:, k * ROW:(k + 1) * ROW],
                        out_offset=None,
                        in_=table_h[:, :],
                        in_offset=bass.IndirectOffsetOnAxis(
                            ap=idxall[:, co + k:co + k + 1], axis=0),
                    )
                G3 = G[:, 0:S * ROW].rearrange("p (s w) -> p s w", w=ROW)
                f1c = G3[:, K:K + 1, D:D + 1]          # [128,1,1] self f1
                f2v = G3[:, 0:K, D + 1:D + 2]          # [128,K,1] edge f2
                lr = lpool.tile([P, Kmax], f32, tag="lr")
                nc.scalar.activation(out=lr[:, 0:K], in_=f2v, func=AF.Identity,
                                     bias=f1c, scale=1.0)
                lr2 = lpool.tile([P, Kmax], f32, tag="lr2")
                nc.vector.tensor_scalar(out=lr2[:, 0:K], in0=lr[:, 0:K],
                                        scalar1=LRELU_ALPHA, scalar2=None,
                                        op0=OP.mult)
                nc.vector.tensor_tensor(out=lr[:, 0:K], in0=lr[:, 0:K],
                                        in1=lr2[:, 0:K], op=OP.max)
                ee = epool.tile([P, Kmax], f16, tag="ee")
                ssum = spool.tile([P, 1], f32, tag="ssum")
                nc.scalar.activation(out=ee[:, 0:K], in_=lr[:, 0:K],
                                     func=AF.Exp, bias=esh_sb[:, 0:1],
                                     scale=1.0, accum_out=ssum[:])
                s2 = spool.tile([P, 1], f32, tag="s2")
                nc.vector.tensor_scalar(out=s2[:], in0=ssum[:], scalar1=1e-30,
                                        scalar2=None, op0=OP.add)
                rec = spool.tile([P, 1], f32, tag="rec")
                nc.vector.reciprocal(out=rec[:], in_=s2[:])

                # one broadcast multiply in place on the feature sub-rows
                gk = G3[:, 0:K, 0:D]
                eb = ee[:, 0:K].rearrange("p (s o) -> p s o", o=1) \
                    .to_broadcast([P, K, D])
                nc.vector.tensor_tensor(out=gk, in0=gk, in1=eb, op=OP.mult)

                # fold-halves tree over slots (fp16, in place on G)
                nsl = K
                while nsl > 2:
                    h = nsl // 2
                    lo = nsl - h
                    nc.vector.tensor_tensor(out=G3[:, 0:h, 0:D],
                                            in0=G3[:, 0:h, 0:D],
                                            in1=G3[:, lo:nsl, 0:D], op=OP.add)
                    nsl = lo
                red = rpool.tile([P, D], f32, tag="red")
                if nsl == 2:
                    nc.vector.tensor_tensor(out=red[:], in0=G3[:, 0:1, 0:D],
                                            in1=G3[:, 1:2, 0:D], op=OP.add)
                else:
                    nc.vector.tensor_copy(out=red[:], in_=G3[:, 0:1, 0:D])

                on = opool.tile([P, D], f16, tag="on")
                nc.vector.tensor_scalar(out=on[:], in0=red[:],
                                        scalar1=rec[:, 0:1], scalar2=None,
                                        op0=OP.mult)
                ob = opool.tile([P, D], f16, tag="ob")
                nc.vector.tensor_tensor(out=ob[:], in0=on[:], in1=bias_sb[:],
                                        op=OP.add)

                # fused elu(elu(x)) = max(x, exp(exp(min(x,0)) - 1) - 1)
                mm = opool.tile([P, D], f16, tag="mm")
                nc.vector.tensor_scalar(out=mm[:], in0=ob[:], scalar1=0.0,
                                        scalar2=None, op0=OP.min)
                ex = opool.tile([P, D], f16, tag="ex")
                nc.scalar.activation(out=ex[:], in_=mm[:], func=AF.Exp,
                                     bias=zero_sb[:, 0:1], scale=1.0)
                ex2 = opool.tile([P, D], f16, tag="ex2")
                nc.scalar.activation(out=ex2[:], in_=ex[:], func=AF.Exp,
                                     bias=negone_sb[:, 0:1], scale=1.0)
                e1 = opool.tile([P, D], f16, tag="e1")
                nc.vector.tensor_scalar(out=e1[:], in0=ex2[:], scalar1=-1.0,
                                        scalar2=None, op0=OP.add)
                fin = fpool2.tile([P, D], f32, tag="fin")
                nc.vector.tensor_tensor(out=fin[:], in0=ob[:], in1=e1[:],
                                        op=OP.max)
                nc.sync.dma_start(out=out_h[r * P:(r + 1) * P, :], in_=fin[:])

    nc.compile()
    return nc


def _run_kernel(X, edge_index, W, a1, b1, a2, b2, bias,
                n=N_NODES, in_dim=IN_DIM, trace=False):
    from concourse.bass_utils import run_bass_kernel_spmd

    dst = np.asarray(edge_index[0], dtype=np.int64)
    src = np.asarray(edge_index[1], dtype=np.int64)
    npad = _ceil_to(n, NCORES * P * 4)  # divisible by 1024 and 512
    order, Kr, coffs, CW, idx_all = _preprocess(dst, src, n, npad)

    exp_shift = 4.0 + max(0.0, float(b1) + float(b2))

    Xp = np.zeros((npad, in_dim), dtype=np.float32)
    Xp[:n] = X
    xt16 = np.ascontiguousarray(Xp[order].T.astype(np.float16))
    w1 = W.astype(np.float64) @ a1.astype(np.float64)
    w2 = W.astype(np.float64) @ a2.astype(np.float64)
    waug = np.concatenate(
        [W.astype(np.float32), w1[:, None].astype(np.float32),
         w2[:, None].astype(np.float32)], axis=1)
    waug16 = np.ascontiguousarray(waug.astype(np.float16))
    brow = np.zeros((1, ROW), dtype=np.float32)
    brow[0, OUT_DIM] = b1
    brow[0, OUT_DIM + 1] = b2
    bias1 = np.ascontiguousarray(bias.astype(np.float32).reshape(1, OUT_DIM))
    dummy = np.zeros((1, ROW), dtype=np.float16)
    dummy[0, OUT_DIM + 1] = DUMMY_F2

    nc = _build_program(npad, Kr, coffs, CW, in_dim, exp_shift)

    in_maps = []
    for c in range(NCORES):
        in_maps.append({
            "xt": xt16, "waug": waug16, "brow": brow, "bias1": bias1,
            "dumrow": dummy, "sidx": np.ascontiguousarray(idx_all[c]),
        })
    res = run_bass_kernel_spmd(nc, in_maps, list(range(NCORES)), trace=trace)
    _last_results["exec_time_ns"] = res.exec_time_ns
    _last_results["mean_exec_time_ns"] = res.mean_exec_time_ns
    _last_results["per_core_scope_times"] = res.per_core_scope_times

    R = len(Kr)
    out_full = np.empty((npad, OUT_DIM), dtype=np.float32)
    rr = np.repeat(np.arange(R), P)
    pp = np.tile(np.arange(P), R)
    for c in range(NCORES):
        pos = (rr * NCORES + c) * P + pp
        out_full[pos] = res.results[c]["out"]
    final = np.empty((npad, OUT_DIM), dtype=np.float32)
    final[order] = out_full
    return np.ascontiguousarray(final[:n])


def kernel(X, edge_index, W, a1, b1, a2, b2, bias):
    trace = bool(int(os.environ.get("GAT_KERNEL_TRACE", "0")))
    return _run_kernel(np.asarray(X, np.float32), np.asarray(edge_index),
                       np.asarray(W, np.float32),
                       np.asarray(a1, np.float32), np.float32(b1),
                       np.asarray(a2, np.float32), np.float32(b2),
                       np.asarray(bias, np.float32), trace=trace)


# revision 21
# speedup vs baseline: 1.2597x; 1.2597x over previous
"""GAT layer kernel for Trainium2 (8 NeuronCores, Bass/Tile).

Strategy:
  - Nodes are permuted by in-degree (host-side index preprocessing) so that
    128-node tiles have homogeneous degree; tiles are dealt round-robin to the
    8 cores so per-core edge counts balance and all cores share one SPMD
    instruction stream (per-tile padded degree K_r identical across cores).
  - Phase A (on device): table[n] = [seq_fts(n) (128, fp16) | f1(n)+b1 |
    f2(n)+b2] built with one PE matmul chain per 128-node tile against the
    host-side augmented weight matrix W_aug = [W | W@a1 | W@a2]; bias adds
    fused into the psum->fp16 copy; table written to HBM 4 tiles per DMA.
  - Phase B (on device): per node-tile, one indirect DMA per padded edge slot
    (plus a self slot providing f1) gathers table rows into a dense
    [node-partition x slot] SBUF layout.  SWDGE descriptor emission on the
    gpsimd Q7 (~8ns/row) is the hard bottleneck, so everything else hides
    under it.  Softmax over slots is free-dim work: ACT lrelu pieces -> ACT
    Exp with accum_out (denominator for free) -> one broadcast tensor_tensor
    multiply in place on the gathered tile -> fold-halves fp16 add tree ->
    normalize-after-aggregation -> fused elu(elu(x)).
    Dummy slots point at a table row with f2 = -60000 so exp() == 0 exactly.
"""

import os
import numpy as np

# ---- problem constants (self-contained; must match reference.py) ----
N_NODES = 100000
N_EDGES = 1600000
IN_DIM = 256
OUT_DIM = 128
LRELU_ALPHA = 0.2

NCORES = 8
P = 128
ROW = OUT_DIM + 2  # 128 feats + f1 + f2
DUMMY_F2 = -60000.0

_last_results = {}


def _ceil_to(x, m):
    return (x + m - 1) // m * m


def _preprocess(dst, src, n, npad):
    """Pure index preprocessing: degree-sort permutation, per-round padded
    degree K_r (exact max), and per-core gather index arrays."""
    ntiles = npad // P
    R = ntiles // NCORES

    deg = np.bincount(dst, minlength=npad).astype(np.int64)
    order = np.argsort(-deg, kind="stable")          # permuted pos -> node
    invpos = np.empty(npad, dtype=np.int64)
    invpos[order] = np.arange(npad)                  # node -> permuted pos

    posdeg = deg[order]                              # descending
    Kr = posdeg[np.arange(R) * (NCORES * P)].astype(np.int64)
    Sr = np.where(Kr > 0, Kr + 1, 0)                 # + self slot if nonempty
    offs = np.zeros(R + 1, dtype=np.int64)
    np.cumsum(P * Sr, out=offs[1:])
    TOT = int(offs[-1])

    # slot index of each edge within its destination node
    pos_d = invpos[dst]
    ordE = np.argsort(pos_d, kind="stable")
    pd_s = pos_d[ordE]
    sp_s = invpos[src][ordE]
    _, first, counts = np.unique(pd_s, return_index=True, return_counts=True)
    slot = np.arange(len(pd_s), dtype=np.int64) - np.repeat(first, counts)

    # SBUF-friendly layout: [128, CW] where column coff[r]+k = round r slot k
    coffs = np.zeros(R + 1, dtype=np.int64)
    np.cumsum(Sr, out=coffs[1:])
    CW = int(coffs[-1])

    g = pd_s >> 7
    p = pd_s & 127
    c = (g % NCORES).astype(np.int64)
    r = g // NCORES
    col = coffs[r] + slot

    idx_all = np.full((NCORES, P, max(CW, 1)), npad, dtype=np.int32)
    idx_all[c, p, col] = sp_s.astype(np.int32)

    # self slots: column coff[r] + K_r = own permuted position
    ne = Sr > 0
    rr = np.repeat(np.arange(R)[ne], P)
    pp = np.tile(np.arange(P), int(ne.sum()))
    self_col = coffs[rr] + Kr[rr]
    for cc in range(NCORES):
        own_pos = (rr * NCORES + cc) * P + pp
        idx_all[cc, pp, self_col] = own_pos.astype(np.int32)

    return order, Kr.tolist(), coffs, CW, idx_all


def _build_program(npad, Kr, coffs, CW, in_dim, exp_shift):
    import concourse.bass as bass
    import concourse.tile as tile
    from concourse import bacc, mybir
    from contextlib import ExitStack

    f16 = mybir.dt.float16
    f32 = mybir.dt.float32
    i32 = mybir.dt.int32
    AF = mybir.ActivationFunctionType
    OP = mybir.AluOpType
    D = OUT_DIM
    KT = in_dim // P
    R = len(Kr)
    rows_per_core = R * P

    nc = bacc.Bacc("TRN2", target_bir_lowering=False, debug=False,
                   num_devices=NCORES)
    xt_h = nc.declare_dram_parameter("xt", [in_dim, npad], f16, isOutput=False)
    waug_h = nc.declare_dram_parameter("waug", [in_dim, ROW], f16,
                                       isOutput=False)
    brow_h = nc.declare_dram_parameter("brow", [1, ROW], f32, isOutput=False)
    bias_h = nc.declare_dram_parameter("bias1", [1, D], f32, isOutput=False)
    dummy_h = nc.declare_dram_parameter("dumrow", [1, ROW], f16, isOutput=False)
    sidx_h = nc.declare_dram_parameter("sidx", [P, max(CW, 1)], i32,
                                       isOutput=False)
    out_h = nc.declare_dram_parameter("out", [rows_per_core, D], f32,
                                      isOutput=True)

    table_h = nc.dram_tensor("table", [npad + 1, ROW], f16)

    with tile.TileContext(nc) as tc, ExitStack() as ctx:
        cpool = ctx.enter_context(tc.tile_pool(name="consts", bufs=1))
        w_sb = [cpool.tile([P, ROW], f16, name=f"wsb{k}", tag=f"w{k}")
                for k in range(KT)]
        for k in range(KT):
            nc.sync.dma_start(out=w_sb[k][:], in_=waug_h[k * P:(k + 1) * P, :])
        brow_sb = cpool.tile([P, ROW], f32, tag="brow")
        nc.sync.dma_start(out=brow_sb[:],
                          in_=brow_h[0:1, :].to_broadcast([P, ROW]))
        bias_sb = cpool.tile([P, D], f32, tag="bias128")
        nc.sync.dma_start(out=bias_sb[:],
                          in_=bias_h[0:1, :].to_broadcast([P, D]))
        esh_sb = cpool.tile([P, 1], f32, tag="eshcol")
        nc.vector.memset(esh_sb[:], -float(exp_shift))
        zero_sb = cpool.tile([P, 1], f32, tag="zerocol")
        nc.vector.memset(zero_sb[:], 0.0)
        negone_sb = cpool.tile([P, 1], f32, tag="negonecol")
        nc.vector.memset(negone_sb[:], -1.0)
        dm_sb = cpool.tile([1, ROW], f16, tag="dummy")
        nc.sync.dma_start(out=dm_sb[:], in_=dummy_h[:, :])

        # fin0 = elu(elu(bias)) for empty rounds
        ob0 = cpool.tile([P, D], f16, tag="ob0")
        nc.vector.tensor_copy(out=ob0[:], in_=bias_sb[:])
        mm0 = cpool.tile([P, D], f16, tag="mm0")
        nc.vector.tensor_scalar(out=mm0[:], in0=ob0[:], scalar1=0.0,
                                scalar2=None, op0=OP.min)
        ex0 = cpool.tile([P, D], f16, tag="ex0")
        nc.scalar.activation(out=ex0[:], in_=mm0[:], func=AF.Exp,
                             bias=zero_sb[:, 0:1], scale=1.0)
        ex20 = cpool.tile([P, D], f16, tag="ex20")
        nc.scalar.activation(out=ex20[:], in_=ex0[:], func=AF.Exp,
                             bias=negone_sb[:, 0:1], scale=1.0)
        e10 = cpool.tile([P, D], f16, tag="e10")
        nc.vector.tensor_scalar(out=e10[:], in0=ex20[:], scalar1=-1.0,
                                scalar2=None, op0=OP.add)
        fin0 = cpool.tile([P, D], f32, tag="fin0")
        nc.vector.tensor_tensor(out=fin0[:], in0=ob0[:], in1=e10[:], op=OP.max)

        # ---------------- Phase A: build the table (node-major) ----------
        with nc.named_scope("phaseA"), ExitStack() as actx:
            xpool = actx.enter_context(tc.tile_pool(name="x", bufs=6))
            pspool = actx.enter_context(
                tc.tile_pool(name="psA", bufs=6, space="PSUM"))
            vpool = actx.enter_context(tc.tile_pool(name="vtile", bufs=4))

            ntile_all = npad // P
            for tb in range(ntile_all // 4):
                xks = []
                for k in range(KT):
                    xk = xpool.tile([P, 4 * P], f16, tag="xk", name=f"xk{k}")
                    nc.scalar.dma_start(
                        out=xk[:],
                        in_=xt_h[k * P:(k + 1) * P, tb * 4 * P:(tb + 1) * 4 * P])
                    xks.append(xk)
                vt4 = vpool.tile([P, 4 * ROW], f16, tag="vt4")
                for j in range(4):
                    ps = pspool.tile([P, ROW], f32, tag="ps")
                    for k in range(KT):
                        lhs = xks[k][:, j * P:(j + 1) * P]
                        nc.tensor.matmul(out=ps[:], lhsT=lhs, rhs=w_sb[k][:],
                                         start=(k == 0), stop=(k == KT - 1))
                    nc.vector.tensor_tensor(out=vt4[:, j * ROW:(j + 1) * ROW],
                                            in0=ps[:], in1=brow_sb[:],
                                            op=OP.add)
                nc.sync.dma_start(
                    out=table_h[tb * 4 * P:(tb + 1) * 4 * P, :].rearrange(
                        "(j p) w -> p j w", p=P),
                    in_=vt4[:].rearrange("p (j w) -> p j w", w=ROW))

        nc.sync.dma_start(out=table_h[npad:npad + 1, :], in_=dm_sb[:])
        tc.strict_bb_all_engine_barrier()

        # ---------------- Phase B: per node-tile edge processing ---------
        with nc.named_scope("phaseB"), ExitStack() as bctx:
            Kmax = max(Kr)
            Smax = Kmax + 1
            gbytes = Smax * ROW * 2
            gbufs = max(2, min(4, 110 * 1024 // gbytes))
            ipool = bctx.enter_context(tc.tile_pool(name="idx", bufs=4))
            gpool = bctx.enter_context(tc.tile_pool(name="g", bufs=gbufs))
            lpool = bctx.enter_context(tc.tile_pool(name="lr", bufs=3))
            epool = bctx.enter_context(tc.tile_pool(name="ee", bufs=3))
            spool = bctx.enter_context(tc.tile_pool(name="small", bufs=8))
            rpool = bctx.enter_context(tc.tile_pool(name="red", bufs=3))
            opool = bctx.enter_context(tc.tile_pool(name="on", bufs=4))
            fpool2 = bctx.enter_context(tc.tile_pool(name="fin", bufs=3))

            for r in range(R):
                K = Kr[r]
                if K == 0:
                    nc.sync.dma_start(out=out_h[r * P:(r + 1) * P, :],
                                      in_=fin0[:])
                    continue
                S = K + 1
                co = int(coffs[r])
                idxt = ipool.tile([P, Smax], i32, tag="idx")
                nc.sync.dma_start(out=idxt[:, 0:S],
                                  in_=sidx_h[:, co:co + S])
                G = gpool.tile([P, Smax * ROW], f16, tag="g")
                for k in range(S):
                    nc.gpsimd.indirect_dma_start(
                        out=G[:, k * ROW:(k + 1) * ROW],
                        out_offset=None,
                        in_=table_h[:, :],
                        in_offset=bass.IndirectOffsetOnAxis(
                            ap=idxt[:, k:k + 1], axis=0),
                    )
                G3 = G[:, 0:S * ROW].rearrange("p (s w) -> p s w", w=ROW)
                f1c = G3[:, K:K + 1, D:D + 1]          # [128,1,1] self f1
                f2v = G3[:, 0:K, D + 1:D + 2]          # [128,K,1] edge f2
                lr = lpool.tile([P, Kmax], f32, tag="lr")
                nc.scalar.activation(out=lr[:, 0:K], in_=f2v, func=AF.Identity,
                                     bias=f1c, scale=1.0)
                lr2 = lpool.tile([P, Kmax], f32, tag="lr2")
                nc.vector.tensor_scalar(out=lr2[:, 0:K], in0=lr[:, 0:K],
                                        scalar1=LRELU_ALPHA, scalar2=None,
                                        op0=OP.mult)
                nc.vector.tensor_tensor(out=lr[:, 0:K], in0=lr[:, 0:K],
                                        in1=lr2[:, 0:K], op=OP.max)
                ee = epool.tile([P, Kmax], f16, tag="ee")
                ssum = spool.tile([P, 1], f32, tag="ssum")
                nc.scalar.activation(out=ee[:, 0:K], in_=lr[:, 0:K],
                                     func=AF.Exp, bias=esh_sb[:, 0:1],
                                     scale=1.0, accum_out=ssum[:])
                s2 = spool.tile([P, 1], f32, tag="s2")
                nc.vector.tensor_scalar(out=s2[:], in0=ssum[:], scalar1=1e-30,
                                        scalar2=None, op0=OP.add)
                rec = spool.tile([P, 1], f32, tag="rec")
                nc.vector.reciprocal(out=rec[:], in_=s2[:])

                # one broadcast multiply in place on the feature sub-rows
                gk = G3[:, 0:K, 0:D]
                eb = ee[:, 0:K].rearrange("p (s o) -> p s o", o=1) \
                    .to_broadcast([P, K, D])
                nc.vector.tensor_tensor(out=gk, in0=gk, in1=eb, op=OP.mult)

                # fold-halves tree over slots (fp16, in place on G)
                nsl = K
                while nsl > 2:
                    h = nsl // 2
                    lo = nsl - h
                    nc.vector.tensor_tensor(out=G3[:, 0:h, 0:D],
                                            in0=G3[:, 0:h, 0:D],
                                            in1=G3[:, lo:nsl, 0:D], op=OP.add)
                    nsl = lo
                red = rpool.tile([P, D], f32, tag="red")
                if nsl == 2:
                    nc.vector.tensor_tensor(out=red[:], in0=G3[:, 0:1, 0:D],
                                            in1=G3[:, 1:2, 0:D], op=OP.add)
                else:
                    nc.vector.tensor_copy(out=red[:], in_=G3[:, 0:1, 0:D])

                on = opool.tile([P, D], f16, tag="on")
                nc.vector.tensor_scalar(out=on[:], in0=red[:],
                                        scalar1=rec[:, 0:1], scalar2=None,
                                        op0=OP.mult)
                ob = opool.tile([P, D], f16, tag="ob")
                nc.vector.tensor_tensor(out=ob[:], in0=on[:], in1=bias_sb[:],
                                        op=OP.add)

                # fused elu(elu(x)) = max(x, exp(exp(min(x,0)) - 1) - 1)
                mm = opool.tile([P, D], f16, tag="mm")
                nc.vector.tensor_scalar(out=mm[:], in0=ob[:], scalar1=0.0,
                                        scalar2=None, op0=OP.min)
                ex = opool.tile([P, D], f16, tag="ex")
                nc.scalar.activation(out=ex[:], in_=mm[:], func=AF.Exp,
                                     bias=zero_sb[:, 0:1], scale=1.0)
                ex2 = opool.tile([P, D], f16, tag="ex2")
                nc.scalar.activation(out=ex2[:], in_=ex[:], func=AF.Exp,
                                     bias=negone_sb[:, 0:1], scale=1.0)
                e1 = opool.tile([P, D], f16, tag="e1")
                nc.vector.tensor_scalar(out=e1[:], in0=ex2[:], scalar1=-1.0,
                                        scalar2=None, op0=OP.add)
                fin = fpool2.tile([P, D], f32, tag="fin")
                nc.vector.tensor_tensor(out=fin[:], in0=ob[:], in1=e1[:],
                                        op=OP.max)
                nc.sync.dma_start(out=out_h[r * P:(r + 1) * P, :], in_=fin[:])

    nc.compile()
    return nc


def _run_kernel(X, edge_index, W, a1, b1, a2, b2, bias,
                n=N_NODES, in_dim=IN_DIM, trace=False):
    from concourse.bass_utils import run_bass_kernel_spmd

    dst = np.asarray(edge_index[0], dtype=np.int64)
    src = np.asarray(edge_index[1], dtype=np.int64)
    npad = _ceil_to(n, NCORES * P * 4)  # divisible by 1024 and 512
    order, Kr, coffs, CW, idx_all = _preprocess(dst, src, n, npad)

    exp_shift = 4.0 + max(0.0, float(b1) + float(b2))

    Xp = np.zeros((npad, in_dim), dtype=np.float32)
    Xp[:n] = X
    xt16 = np.ascontiguousarray(Xp[order].T.astype(np.float16))
    w1 = W.astype(np.float64) @ a1.astype(np.float64)
    w2 = W.astype(np.float64) @ a2.astype(np.float64)
    waug = np.concatenate(
        [W.astype(np.float32), w1[:, None].astype(np.float32),
         w2[:, None].astype(np.float32)], axis=1)
    waug16 = np.ascontiguousarray(waug.astype(np.float16))
    brow = np.zeros((1, ROW), dtype=np.float32)
    brow[0, OUT_DIM] = b1
    brow[0, OUT_DIM + 1] = b2
    bias1 = np.ascontiguousarray(bias.astype(np.float32).reshape(1, OUT_DIM))
    dummy = np.zeros((1, ROW), dtype=np.float16)
    dummy[0, OUT_DIM + 1] = DUMMY_F2

    nc = _build_program(npad, Kr, coffs, CW, in_dim, exp_shift)

    in_maps = []
    for c in range(NCORES):
        in_maps.append({
            "xt": xt16, "waug": waug16, "brow": brow, "bias1": bias1,
            "dumrow": dummy, "sidx": np.ascontiguousarray(idx_all[c]),
        })
    res = run_bass_kernel_spmd(nc, in_maps, list(range(NCORES)), trace=trace)
    _last_results["exec_time_ns"] = res.exec_time_ns
    _last_results["mean_exec_time_ns"] = res.mean_exec_time_ns
    _last_results["per_core_scope_times"] = res.per_core_scope_times

    R = len(Kr)
    out_full = np.empty((npad, OUT_DIM), dtype=np.float32)
    rr = np.repeat(np.arange(R), P)
    pp = np.tile(np.arange(P), R)
    for c in range(NCORES):
        pos = (rr * NCORES + c) * P + pp
        out_full[pos] = res.results[c]["out"]
    final = np.empty((npad, OUT_DIM), dtype=np.float32)
    final[order] = out_full
    return np.ascontiguousarray(final[:n])


def kernel(X, edge_index, W, a1, b1, a2, b2, bias):
    trace = bool(int(os.environ.get("GAT_KERNEL_TRACE", "0")))
    return _run_kernel(np.asarray(X, np.float32), np.asarray(edge_index),
                       np.asarray(W, np.float32),
                       np.asarray(a1, np.float32), np.float32(b1),
                       np.asarray(a2, np.float32), np.float32(b2),
                       np.asarray(bias, np.float32), trace=trace)


# revision 40
# speedup vs baseline: 1.2950x; 1.0280x over previous
"""GAT layer kernel for Trainium2 (8 NeuronCores, Bass/Tile).

Strategy:
  - Nodes are permuted by in-degree (host-side index preprocessing) so that
    128-node tiles have homogeneous degree; tiles are dealt round-robin to the
    8 cores so per-core edge counts balance and all cores share one SPMD
    instruction stream (per-tile padded degree K_r identical across cores).
  - Phase A (on device): table[n] = [seq_fts(n) (128, fp16) | f1(n)+b1 |
    f2(n)+b2] built with one PE matmul chain per 128-node tile against the
    host-side augmented weight matrix W_aug = [W | W@a1 | W@a2]; bias adds
    fused into the psum->fp16 copy; table written to HBM 4 tiles per DMA.
  - Phase B (on device): per node-tile, one indirect DMA per padded edge slot
    (plus a self slot providing f1) gathers table rows into a dense
    [node-partition x slot] SBUF layout.  SWDGE descriptor emission on the
    gpsimd Q7 (~8ns/row) is the hard bottleneck, so everything else hides
    under it.  Softmax over slots is free-dim work: ACT lrelu pieces -> ACT
    Exp with accum_out (denominator for free) -> one broadcast tensor_tensor
    multiply in place on the gathered tile -> fold-halves fp16 add tree ->
    normalize-after-aggregation -> fused elu(elu(x)).
    Dummy slots point at a table row with f2 = -60000 so exp() == 0 exactly.
"""

import os
import numpy as np

# ---- problem constants (self-contained; must match reference.py) ----
N_NODES = 100000
N_EDGES = 1600000
IN_DIM = 256
OUT_DIM = 128
LRELU_ALPHA = 0.2

NCORES = 8
P = 128
ROW = OUT_DIM + 2  # 128 feats + f1 + f2
DUMMY_F2 = -60000.0

_last_results = {}


def _ceil_to(x, m):
    return (x + m - 1) // m * m


def _preprocess(dst, src, n, npad):
    """Pure index preprocessing: degree-sort permutation, per-round padded
    degree K_r (exact max), and per-core gather index arrays."""
    ntiles = npad // P
    R = ntiles // NCORES

    deg = np.bincount(dst, minlength=npad).astype(np.int64)
    order = np.argsort(-deg, kind="stable")          # permuted pos -> node
    invpos = np.empty(npad, dtype=np.int64)
    invpos[order] = np.arange(npad)                  # node -> permuted pos

    posdeg = deg[order]                              # descending
    Kr = posdeg[np.arange(R) * (NCORES * P)].astype(np.int64)
    Sr = np.where(Kr > 0, Kr + 1, 0)                 # + self slot if nonempty
    offs = np.zeros(R + 1, dtype=np.int64)
    np.cumsum(P * Sr, out=offs[1:])
    TOT = int(offs[-1])

    # slot index of each edge within its destination node
    pos_d = invpos[dst]
    ordE = np.argsort(pos_d, kind="stable")
    pd_s = pos_d[ordE]
    sp_s = invpos[src][ordE]
    _, first, counts = np.unique(pd_s, return_index=True, return_counts=True)
    slot = np.arange(len(pd_s), dtype=np.int64) - np.repeat(first, counts)

    # SBUF-friendly layout: [128, CW] where column coff[r]+k = round r slot k
    coffs = np.zeros(R + 1, dtype=np.int64)
    np.cumsum(Sr, out=coffs[1:])
    CW = int(coffs[-1])

    g = pd_s >> 7
    p = pd_s & 127
    c = (g % NCORES).astype(np.int64)
    r = g // NCORES
    col = coffs[r] + slot

    idx_all = np.full((NCORES, P, max(CW, 1)), npad, dtype=np.int32)
    idx_all[c, p, col] = sp_s.astype(np.int32)

    # self slots: column coff[r] + K_r = own permuted position
    ne = Sr > 0
    rr = np.repeat(np.arange(R)[ne], P)
    pp = np.tile(np.arange(P), int(ne.sum()))
    self_col = coffs[rr] + Kr[rr]
    for cc in range(NCORES):
        own_pos = (rr * NCORES + cc) * P + pp
        idx_all[cc, pp, self_col] = own_pos.astype(np.int32)

    return order, Kr.tolist(), coffs, CW, idx_all


def _build_program(npad, Kr, coffs, CW, in_dim, exp_shift):
    import concourse.bass as bass
    import concourse.tile as tile
    from concourse import bacc, mybir
    from contextlib import ExitStack

    f16 = mybir.dt.float16
    f32 = mybir.dt.float32
    i32 = mybir.dt.int32
    AF = mybir.ActivationFunctionType
    OP = mybir.AluOpType
    D = OUT_DIM
    KT = in_dim // P
    R = len(Kr)
    rows_per_core = R * P

    nc = bacc.Bacc("TRN2", target_bir_lowering=False, debug=False,
                   num_devices=NCORES)
    xt_h = nc.declare_dram_parameter("xt", [in_dim, npad], f16, isOutput=False)
    waug_h = nc.declare_dram_parameter("waug", [in_dim, ROW], f16,
                                       isOutput=False)
    brow_h = nc.declare_dram_parameter("brow", [1, ROW], f32, isOutput=False)
    bias_h = nc.declare_dram_parameter("bias1", [1, D], f32, isOutput=False)
    dummy_h = nc.declare_dram_parameter("dumrow", [1, ROW], f16, isOutput=False)
    sidx_h = nc.declare_dram_parameter("sidx", [P, max(CW, 1)], i32,
                                       isOutput=False)
    out_h = nc.declare_dram_parameter("out", [rows_per_core, D], f32,
                                      isOutput=True)

    table_h = nc.dram_tensor("table", [npad + 1, ROW], f16)

    with tile.TileContext(nc) as tc, ExitStack() as ctx:
        cpool = ctx.enter_context(tc.tile_pool(name="consts", bufs=1))
        w_sb = [cpool.tile([P, ROW], f16, name=f"wsb{k}", tag=f"w{k}")
                for k in range(KT)]
        for k in range(KT):
            nc.sync.dma_start(out=w_sb[k][:], in_=waug_h[k * P:(k + 1) * P, :])
        brow_sb = cpool.tile([P, ROW], f32, tag="brow")
        nc.sync.dma_start(out=brow_sb[:],
                          in_=brow_h[0:1, :].to_broadcast([P, ROW]))
        bias_sb = cpool.tile([P, D], f32, tag="bias128")
        nc.sync.dma_start(out=bias_sb[:],
                          in_=bias_h[0:1, :].to_broadcast([P, D]))
        esh_sb = cpool.tile([P, 1], f32, tag="eshcol")
        nc.vector.memset(esh_sb[:], -float(exp_shift))
        zero_sb = cpool.tile([P, 1], f32, tag="zerocol")
        nc.vector.memset(zero_sb[:], 0.0)
        negone_sb = cpool.tile([P, 1], f32, tag="negonecol")
        nc.vector.memset(negone_sb[:], -1.0)
        dm_sb = cpool.tile([1, ROW], f16, tag="dummy")
        nc.sync.dma_start(out=dm_sb[:], in_=dummy_h[:, :])

        # fin0 = elu(elu(bias)) for empty rounds
        ob0 = cpool.tile([P, D], f16, tag="ob0")
        nc.vector.tensor_copy(out=ob0[:], in_=bias_sb[:])
        mm0 = cpool.tile([P, D], f16, tag="mm0")
        nc.vector.tensor_scalar(out=mm0[:], in0=ob0[:], scalar1=0.0,
                                scalar2=None, op0=OP.min)
        ex0 = cpool.tile([P, D], f16, tag="ex0")
        nc.scalar.activation(out=ex0[:], in_=mm0[:], func=AF.Exp,
                             bias=zero_sb[:, 0:1], scale=1.0)
        ex20 = cpool.tile([P, D], f16, tag="ex20")
        nc.scalar.activation(out=ex20[:], in_=ex0[:], func=AF.Exp,
                             bias=negone_sb[:, 0:1], scale=1.0)
        e10 = cpool.tile([P, D], f16, tag="e10")
        nc.vector.tensor_scalar(out=e10[:], in0=ex20[:], scalar1=-1.0,
                                scalar2=None, op0=OP.add)
        fin0 = cpool.tile([P, D], f32, tag="fin0")
        nc.vector.tensor_tensor(out=fin0[:], in0=ob0[:], in1=e10[:], op=OP.max)

        # ---------------- Phase A: build the table (node-major) ----------
        with nc.named_scope("phaseA"), ExitStack() as actx:
            xpool = actx.enter_context(tc.tile_pool(name="x", bufs=6))
            pspool = actx.enter_context(
                tc.tile_pool(name="psA", bufs=6, space="PSUM"))
            vpool = actx.enter_context(tc.tile_pool(name="vtile", bufs=4))

            ntile_all = npad // P
            for tb in range(ntile_all // 4):
                xks = []
                for k in range(KT):
                    xk = xpool.tile([P, 4 * P], f16, tag="xk", name=f"xk{k}")
                    nc.scalar.dma_start(
                        out=xk[:],
                        in_=xt_h[k * P:(k + 1) * P, tb * 4 * P:(tb + 1) * 4 * P])
                    xks.append(xk)
                vt4 = vpool.tile([P, 4 * ROW], f16, tag="vt4")
                for j in range(4):
                    ps = pspool.tile([P, ROW], f32, tag="ps")
                    for k in range(KT):
                        lhs = xks[k][:, j * P:(j + 1) * P]
                        nc.tensor.matmul(out=ps[:], lhsT=lhs, rhs=w_sb[k][:],
                                         start=(k == 0), stop=(k == KT - 1))
                    nc.vector.tensor_tensor(out=vt4[:, j * ROW:(j + 1) * ROW],
                                            in0=ps[:], in1=brow_sb[:],
                                            op=OP.add)
                nc.sync.dma_start(
                    out=table_h[tb * 4 * P:(tb + 1) * 4 * P, :].rearrange(
                        "(j p) w -> p j w", p=P),
                    in_=vt4[:].rearrange("p (j w) -> p j w", w=ROW))

        nc.sync.dma_start(out=table_h[npad:npad + 1, :], in_=dm_sb[:])
        tc.strict_bb_all_engine_barrier()

        # ---------------- Phase B: per node-tile edge processing ---------
        with nc.named_scope("phaseB"), ExitStack() as bctx:
            Kmax = max(Kr)
            Smax = Kmax + 1
            gbytes = Smax * ROW * 2
            gbufs = max(2, min(6, 110 * 1024 // gbytes))
            ipool = bctx.enter_context(tc.tile_pool(name="idx", bufs=6))
            gpool = bctx.enter_context(tc.tile_pool(name="g", bufs=gbufs))
            lpool = bctx.enter_context(tc.tile_pool(name="lr", bufs=3))
            epool = bctx.enter_context(tc.tile_pool(name="ee", bufs=3))
            spool = bctx.enter_context(tc.tile_pool(name="small", bufs=8))
            rpool = bctx.enter_context(tc.tile_pool(name="red", bufs=3))
            opool = bctx.enter_context(tc.tile_pool(name="on", bufs=4))
            fpool2 = bctx.enter_context(tc.tile_pool(name="fin", bufs=3))

            for r in range(R):
                K = Kr[r]
                if K == 0:
                    nc.sync.dma_start(out=out_h[r * P:(r + 1) * P, :],
                                      in_=fin0[:])
                    continue
                S = K + 1
                co = int(coffs[r])
                idxt = ipool.tile([P, Smax], i32, tag="idx")
                nc.sync.dma_start(out=idxt[:, 0:S],
                                  in_=sidx_h[:, co:co + S])
                G = gpool.tile([P, Smax * ROW], f16, tag="g")
                for k in range(S):
                    nc.gpsimd.indirect_dma_start(
                        out=G[:, k * ROW:(k + 1) * ROW],
                        out_offset=None,
                        in_=table_h[:, :],
                        in_offset=bass.IndirectOffsetOnAxis(
                            ap=idxt[:, k:k + 1], axis=0),
                    )
                G3 = G[:, 0:S * ROW].rearrange("p (s w) -> p s w", w=ROW)
                f1c = G3[:, K:K + 1, D:D + 1]          # [128,1,1] self f1
                f2v = G3[:, 0:K, D + 1:D + 2]          # [128,K,1] edge f2
                lr = lpool.tile([P, Kmax], f32, tag="lr")
                nc.scalar.activation(out=lr[:, 0:K], in_=f2v, func=AF.Identity,
                                     bias=f1c, scale=1.0)
                lr2 = lpool.tile([P, Kmax], f32, tag="lr2")
                nc.vector.tensor_scalar(out=lr2[:, 0:K], in0=lr[:, 0:K],
                                        scalar1=LRELU_ALPHA, scalar2=None,
                                        op0=OP.mult)
                nc.vector.tensor_tensor(out=lr[:, 0:K], in0=lr[:, 0:K],
                                        in1=lr2[:, 0:K], op=OP.max)
                ee = epool.tile([P, Kmax], f16, tag="ee")
                ssum = spool.tile([P, 1], f32, tag="ssum")
                nc.scalar.activation(out=ee[:, 0:K], in_=lr[:, 0:K],
                                     func=AF.Exp, bias=esh_sb[:, 0:1],
                                     scale=1.0, accum_out=ssum[:])
                s2 = spool.tile([P, 1], f32, tag="s2")
                nc.vector.tensor_scalar(out=s2[:], in0=ssum[:], scalar1=1e-30,
                                        scalar2=None, op0=OP.add)
                rec = spool.tile([P, 1], f32, tag="rec")
                nc.vector.reciprocal(out=rec[:], in_=s2[:])

                # one broadcast multiply in place on the feature sub-rows
                gk = G3[:, 0:K, 0:D]
                eb = ee[:, 0:K].rearrange("p (s o) -> p s o", o=1) \
                    .to_broadcast([P, K, D])
                nc.vector.tensor_tensor(out=gk, in0=gk, in1=eb, op=OP.mult)

                # fold-halves tree over slots (fp16, in place on G)
                nsl = K
                while nsl > 2:
                    h = nsl // 2
                    lo = nsl - h
                    nc.vector.tensor_tensor(out=G3[:, 0:h, 0:D],
                                            in0=G3[:, 0:h, 0:D],
                                            in1=G3[:, lo:nsl, 0:D], op=OP.add)
                    nsl = lo
                red = rpool.tile([P, D], f32, tag="red")
                if nsl == 2:
                    nc.vector.tensor_tensor(out=red[:], in0=G3[:, 0:1, 0:D],
                                            in1=G3[:, 1:2, 0:D], op=OP.add)
                else:
                    nc.vector.tensor_copy(out=red[:], in_=G3[:, 0:1, 0:D])

                on = opool.tile([P, D], f16, tag="on")
                nc.vector.tensor_scalar(out=on[:], in0=red[:],
                                        scalar1=rec[:, 0:1], scalar2=None,
                                        op0=OP.mult)
                ob = opool.tile([P, D], f16, tag="ob")
                nc.vector.tensor_tensor(out=ob[:], in0=on[:], in1=bias_sb[:],
                                        op=OP.add)

                # fused elu(elu(x)) = max(x, exp(exp(min(x,0)) - 1) - 1)
                mm = opool.tile([P, D], f16, tag="mm")
                nc.vector.tensor_scalar(out=mm[:], in0=ob[:], scalar1=0.0,
                                        scalar2=None, op0=OP.min)
                ex = opool.tile([P, D], f16, tag="ex")
                nc.scalar.activation(out=ex[:], in_=mm[:], func=AF.Exp,
                                     bias=zero_sb[:, 0:1], scale=1.0)
                ex2 = opool.tile([P, D], f16, tag="ex2")
                nc.scalar.activation(out=ex2[:], in_=ex[:], func=AF.Exp,
                                     bias=negone_sb[:, 0:1], scale=1.0)
                e1 = opool.tile([P, D], f16, tag="e1")
                nc.vector.tensor_scalar(out=e1[:], in0=ex2[:], scalar1=-1.0,
                                        scalar2=None, op0=OP.add)
                fin = fpool2.tile([P, D], f32, tag="fin")
                nc.vector.tensor_tensor(out=fin[:], in0=ob[:], in1=e1[:],
                                        op=OP.max)
                nc.sync.dma_start(out=out_h[r * P:(r + 1) * P, :], in_=fin[:])

    nc.compile()
    return nc


def _run_kernel(X, edge_index, W, a1, b1, a2, b2, bias,
                n=N_NODES, in_dim=IN_DIM, trace=False):
    from concourse.bass_utils import run_bass_kernel_spmd

    dst = np.asarray(edge_index[0], dtype=np.int64)
    src = np.asarray(edge_index[1], dtype=np.int64)
    npad = _ceil_to(n, NCORES * P * 4)  # divisible by 1024 and 512
    order, Kr, coffs, CW, idx_all = _preprocess(dst, src, n, npad)

    exp_shift = 4.0 + max(0.0, float(b1) + float(b2))

    Xp = np.zeros((npad, in_dim), dtype=np.float32)
    Xp[:n] = X
    xt16 = np.ascontiguousarray(Xp[order].T.astype(np.float16))
    w1 = W.astype(np.float64) @ a1.astype(np.float64)
    w2 = W.astype(np.float64) @ a2.astype(np.float64)
    waug = np.concatenate(
        [W.astype(np.float32), w1[:, None].astype(np.float32),
         w2[:, None].astype(np.float32)], axis=1)
    waug16 = np.ascontiguousarray(waug.astype(np.float16))
    brow = np.zeros((1, ROW), dtype=np.float32)
    brow[0, OUT_DIM] = b1
    brow[0, OUT_DIM + 1] = b2
    bias1 = np.ascontiguousarray(bias.astype(np.float32).reshape(1, OUT_DIM))
    dummy = np.zeros((1, ROW), dtype=np.float16)
    dummy[0, OUT_DIM + 1] = DUMMY_F2

    nc = _build_program(npad, Kr, coffs, CW, in_dim, exp_shift)

    in_maps = []
    for c in range(NCORES):
        in_maps.append({
            "xt": xt16, "waug": waug16, "brow": brow, "bias1": bias1,
            "dumrow": dummy, "sidx": np.ascontiguousarray(idx_all[c]),
        })
    res = run_bass_kernel_spmd(nc, in_maps, list(range(NCORES)), trace=trace)
    _last_results["exec_time_ns"] = res.exec_time_ns
    _last_results["mean_exec_time_ns"] = res.mean_exec_time_ns
    _last_results["per_core_scope_times"] = res.per_core_scope_times

    R = len(Kr)
    out_full = np.empty((npad, OUT_DIM), dtype=np.float32)
    rr = np.repeat(np.arange(R), P)
    pp = np.tile(np.arange(P), R)
    for c in range(NCORES):
        pos = (rr * NCORES + c) * P + pp
        out_full[pos] = res.results[c]["out"]
    final = np.empty((npad, OUT_DIM), dtype=np.float32)
    final[order] = out_full
    return np.ascontiguousarray(final[:n])


def kernel(X, edge_index, W, a1, b1, a2, b2, bias):
    trace = bool(int(os.environ.get("GAT_KERNEL_TRACE", "0")))
    return _run_kernel(np.asarray(X, np.float32), np.asarray(edge_index),
                       np.asarray(W, np.float32),
                       np.asarray(a1, np.float32), np.float32(b1),
                       np.asarray(a2, np.float32), np.float32(b2),
                       np.asarray(bias, np.float32), trace=trace)
